# revision 3
# baseline (speedup 1.0000x reference)
"""Multi-head attention (B=2, S=2048, D=1024, H=16) on 8 trn2 NeuronCores.

Sharding: core c -> batch b = c // 4, head group g = c % 4 (heads 4g..4g+3).
Each core computes, for its batch shard and 4 heads:
  QT/KT = (x W + b)^T in [d_local, seq] layout, V in [seq, d_local] layout,
  transposed scores S^T[k, q] = K Q^T (so softmax needs no transposes),
  exp via ACT (scale fused), PV matmul with an appended ones column which
  yields both the unnormalized context and the softmax row sums,
  normalization via a gpsimd partition-broadcast reciprocal multiply,
  and a partial output projection against a row shard of Wo.
Host sums the 4 partials per batch and adds the constant row bv @ Wo + bo
(softmax rows sum to one, so bv's contribution is a constant vector).
"""

import sys

sys.path.insert(0, "/opt/trn_rl_repo")

import numpy as np
import ml_dtypes

B = 2
S = 2048
D = 1024
H = 16
HD = 64
NCORES = 8
HPC = 4          # heads per core
DL = HPC * HD    # 256 local head dims per core
P = 128
KCH = S // P     # 16 key chunks
DCH = D // P     # 8 contraction chunks
TBLK = S // P    # 16 token blocks
SCALE = 1.0 / np.sqrt(HD)

_CACHE = {}


def _build():
    import concourse.bass as bass  # noqa: F401
    import concourse.mybir as mybir
    import concourse.tile as tile
    from concourse import bacc

    bf16 = mybir.dt.bfloat16
    f32 = mybir.dt.float32

    nc = bacc.Bacc("TRN2", target_bir_lowering=False, debug=False,
                   num_devices=NCORES)

    xT_d = nc.dram_tensor("xt", [D, S], bf16, kind="ExternalInput")
    wq_d = nc.dram_tensor("wq", [D, DL], bf16, kind="ExternalInput")
    wk_d = nc.dram_tensor("wk", [D, DL], bf16, kind="ExternalInput")
    wv_d = nc.dram_tensor("wv", [D, DL], bf16, kind="ExternalInput")
    wo_d = nc.dram_tensor("wo", [DL, D], bf16, kind="ExternalInput")
    bqk_d = nc.dram_tensor("bqk", [P, 4], f32, kind="ExternalInput")
    out_d = nc.dram_tensor("out", [S, D], f32, kind="ExternalOutput")

    with tile.TileContext(nc) as tc:
        with (
            tc.tile_pool(name="persist", bufs=1) as pp,
            tc.tile_pool(name="stream", bufs=3) as sp,
            tc.tile_pool(name="psum", bufs=2, space="PSUM") as ps,
        ):
            # ---- load inputs ----
            xts = []
            for c in range(DCH):
                xt = pp.tile([P, S], bf16, tag=f"xt{c}", name=f"xt{c}")
                nc.sync.dma_start(xt[:], xT_d[c * P:(c + 1) * P, :])
                xts.append(xt)
            wq_s = pp.tile([P, DCH, DL], bf16, tag="wq", name="wq_s")
            wk_s = pp.tile([P, DCH, DL], bf16, tag="wk", name="wk_s")
            wv_s = pp.tile([P, DCH, DL], bf16, tag="wv", name="wv_s")
            for w_s, w_d in ((wq_s, wq_d), (wk_s, wk_d), (wv_s, wv_d)):
                nc.sync.dma_start(
                    w_s[:], w_d[:].rearrange("(c p) n -> p c n", p=P))
            wo_s = pp.tile([P, 2, D], bf16, tag="wo", name="wo_s")
            nc.sync.dma_start(
                wo_s[:], wo_d[:].rearrange("(c p) n -> p c n", p=P))
            bqk_s = pp.tile([P, 4], f32, tag="bqk", name="bqk_s")
            nc.sync.dma_start(bqk_s[:], bqk_d[:])

            # ---- Q/K projections -> QT/KT [d_local, seq] (bf16) ----
            qkt = []  # [q/k][dblk] -> [128, S] tile
            for wi, (w_s, bcol, nm) in enumerate(
                    ((wq_s, 0, "qt"), (wk_s, 2, "kt"))):
                tiles = []
                for dblk in range(2):
                    t_sb = pp.tile([P, S], bf16, tag=f"{nm}{dblk}",
                                   name=f"{nm}{dblk}")
                    for half in range(2):
                        acc = ps.tile([P, 1024], f32, tag="work",
                                      name=f"ps_{nm}{dblk}_{half}")
                        for kc in range(DCH):
                            for ns in range(2):
                                nc.tensor.matmul(
                                    acc[:, ns * 512:(ns + 1) * 512],
                                    w_s[:, kc, dblk * P:(dblk + 1) * P],
                                    xts[kc][:, half * 1024 + ns * 512:
                                            half * 1024 + (ns + 1) * 512],
                                    start=(kc == 0), stop=(kc == DCH - 1),
                                )
                        nc.vector.tensor_scalar_add(
                            t_sb[:, half * 1024:(half + 1) * 1024],
                            acc[:],
                            bqk_s[:, bcol + dblk:bcol + dblk + 1],
                        )
                    tiles.append(t_sb)
                qkt.append(tiles)
            qt, kt = qkt

            # ---- V projection -> [seq, 4 * 65] with ones columns ----
            # per token block: V_sb[:, h * 65 + j] = V[t, h * 64 + j],
            # V_sb[:, h * 65 + 64] = 1.0
            vts = []
            for tb in range(TBLK):
                vt = pp.tile([P, HPC * 65], bf16, tag=f"v{tb}",
                             name=f"v{tb}")
                v3 = vt.rearrange("p (h e) -> p h e", e=65)
                nc.gpsimd.memset(v3[:, :, 64:65], 1.0)
                acc = ps.tile([P, 1024], f32, tag="work", name=f"ps_v{tb}")
                for kc in range(DCH):
                    nc.tensor.matmul(
                        acc[:, 0:DL],
                        xts[kc][:, tb * P:(tb + 1) * P],
                        wv_s[:, kc, :],
                        start=(kc == 0), stop=(kc == DCH - 1),
                    )
                nc.vector.tensor_copy(
                    v3[:, :, 0:64],
                    acc[:, 0:DL].rearrange("p (h e) -> p h e", e=64),
                )
                vts.append(vt)

            # ---- attention per head ----
            ctx_sb = [pp.tile([P, S], bf16, tag=f"ctx{dc}", name=f"ctx{dc}")
                      for dc in range(2)]
            for h in range(HPC):
                dblk = h // 2
                roff = 64 * (h % 2)
                ctx_ps = ps.tile([P, S], f32, tag="ctx", bufs=1,
                                 name=f"ps_ctx{h}")
                for kc in range(KCH):
                    et = sp.tile([P, S], bf16, tag="expt", name=f"expt{h}_{kc}")
                    for half in range(2):
                        sc = ps.tile([P, 1024], f32, tag="work",
                                     name=f"ps_sc{h}_{kc}_{half}")
                        for ns in range(2):
                            nc.tensor.matmul(
                                sc[:, ns * 512:(ns + 1) * 512],
                                kt[dblk][roff:roff + 64,
                                         kc * P:(kc + 1) * P],
                                qt[dblk][roff:roff + 64,
                                         half * 1024 + ns * 512:
                                         half * 1024 + (ns + 1) * 512],
                                start=True, stop=True,
                            )
                        nc.scalar.activation(
                            et[:, half * 1024:(half + 1) * 1024],
                            sc[:],
                            mybir.ActivationFunctionType.Exp,
                            scale=float(SCALE),
                        )
                    for ns in range(4):
                        nc.tensor.matmul(
                            ctx_ps[0:65, ns * 512:(ns + 1) * 512],
                            vts[kc][:, h * 65:(h + 1) * 65],
                            et[:, ns * 512:(ns + 1) * 512],
                            start=(kc == 0), stop=(kc == KCH - 1),
                        )
                # normalize: ctx[d, q] / s[q]
                rec = sp.tile([1, S], f32, tag="rec", bufs=2, name=f"rec{h}")
                nc.vector.reciprocal(rec[:], ctx_ps[64:65, :])
                bc = sp.tile([64, S], f32, tag="bc", bufs=2, name=f"bc{h}")
                nc.gpsimd.partition_broadcast(bc[:], rec[:])
                nc.vector.tensor_mul(
                    ctx_sb[dblk][roff:roff + 64, :], ctx_ps[0:64, :], bc[:])

            # ---- output projection ----
            for tb in range(TBLK):
                acc = ps.tile([P, 1024], f32, tag="work", name=f"ps_o{tb}")
                for dc in range(2):
                    for ns in range(2):
                        nc.tensor.matmul(
                            acc[:, ns * 512:(ns + 1) * 512],
                            ctx_sb[dc][:, tb * P:(tb + 1) * P],
                            wo_s[:, dc, ns * 512:(ns + 1) * 512],
                            start=(dc == 0), stop=(dc == 1),
                        )
                o_sb = sp.tile([P, D], f32, tag="osb", name=f"osb{tb}")
                nc.vector.tensor_copy(o_sb[:], acc[:])
                nc.sync.dma_start(out_d[tb * P:(tb + 1) * P, :], o_sb[:])

    nc.compile()
    return nc


def _get_compiled():
    if "nc" not in _CACHE:
        _CACHE["nc"] = _build()
    return _CACHE["nc"]


def kernel(x, Wq, bq, Wk, bk, Wv, bv, Wo, bo):
    from concourse.bass_utils import run_bass_kernel_spmd

    nc = _get_compiled()
    x = np.asarray(x, dtype=np.float32)
    Wq, bq = np.asarray(Wq, np.float32), np.asarray(bq, np.float32)
    Wk, bk = np.asarray(Wk, np.float32), np.asarray(bk, np.float32)
    Wv, bv = np.asarray(Wv, np.float32), np.asarray(bv, np.float32)
    Wo, bo = np.asarray(Wo, np.float32), np.asarray(bo, np.float32)

    bf = ml_dtypes.bfloat16
    in_maps = []
    for c in range(NCORES):
        b, g = c // 4, c % 4
        cols = slice(g * DL, (g + 1) * DL)
        bq_l, bk_l = bq[cols], bk[cols]
        bqk = np.stack(
            [bq_l[0:P], bq_l[P:2 * P], bk_l[0:P], bk_l[P:2 * P]], axis=1)
        in_maps.append({
            "xt": np.ascontiguousarray(x[b].T).astype(bf),
            "wq": Wq[:, cols].astype(bf),
            "wk": Wk[:, cols].astype(bf),
            "wv": Wv[:, cols].astype(bf),
            "wo": Wo[cols, :].astype(bf),
            "bqk": np.ascontiguousarray(bqk, np.float32),
        })

    _CACHE["in_maps"] = in_maps
    res = run_bass_kernel_spmd(nc, in_maps, list(range(NCORES)))

    # constant row: bv @ Wo + bo (softmax rows sum to 1)
    const_row = bv.astype(np.float64) @ Wo.astype(np.float64) + bo
    out = np.zeros((B, S, D), np.float64)
    for c in range(NCORES):
        out[c // 4] += res.results[c]["out"].astype(np.float64)
    out += const_row
    return out.astype(np.float32)
